# revision 5
# baseline (speedup 1.0000x reference)
"""Multi-head attention (b=4, n=2048, d=1024, h=16, dh=64) on 8 TRN2 NeuronCores.

Sharding: batch x sequence-half per core (core c handles batch b=c//2, query
rows s=(c%2)*1024 .. +1024). K/V projections are deduplicated across the two
cores sharing a batch: each core projects K/V only for its OWN 1024 rows and
the halves are exchanged with the paired core via a pairwise AllGather
(replica_groups=[[0,1],[2,3],[4,5],[6,7]]) through internal DRAM bounce
buffers. Key order in the gathered KT/V is canonical (rank0 rows then rank1
rows) and identical on both cores, so the SPMD program is uniform; softmax is
key-order invariant as long as KT and V agree.

Phase order is chosen so TensorE never waits on the exchange:
  V-own proj -> CC(V) -> K-own proj -> CC(K) -> Q proj -> attention loop.
The attention loop is ib-outer so the Wo projection for the first 512 queries
overlaps the second half's score/AV work.

exp(softmax) is the ScalarE bottleneck (~1 elem/cycle/lane), so a tunable
fraction of the exp tiles is offloaded to DVE/GPSIMD using the bf16
Schraudolph bit trick: bf16_bits(exp(x)) ~= round(x*SCALE*128*log2(e) + C2)
computed as one tensor_scalar into an int16-aliased view of the es tile
(~1.8% rms per-element error on those tiles only; the rest stay exact on
ScalarE).
"""

import sys

sys.path.insert(0, "/opt/trn_rl_repo")

from contextlib import ExitStack

import numpy as np

import concourse.bass as bass
import concourse.tile as tile
from concourse import bacc, mybir
from concourse.bass_utils import run_bass_kernel_spmd

F32 = mybir.dt.float32
BF16 = mybir.dt.bfloat16
I16 = mybir.dt.int16
EXP = mybir.ActivationFunctionType.Exp

P = 128
D = 1024  # model dim
NI = 1024  # rows per core (queries AND own keys)
NJ = 2048  # total keys per batch
H = 16  # heads
DH = 64  # head dim
SCALE = DH**-0.5  # 0.125
NCORES = 8

NCC = D // P  # 8 contraction chunks
NDB = D // P  # 8 feature blocks (head pairs)

# Schraudolph bf16 exp constants (C2 tuned for balanced relative error with
# round-to-nearest int16 conversion; see micro_exp.py).
EXP_C1 = float(128.0 * np.log2(np.e))
EXP_C2 = 16249.0

# exp-engine policy: for tile index k (0..7 within each (head, ib) group),
# which engine computes it. 'a' = ScalarE exact, 'v' = DVE trick,
# 'p' = GPSIMD trick.
EXP_POLICY = "aavaaava"  # 5 exact + 3 approx per group


def _build():
    nc = bacc.Bacc("TRN2", target_bir_lowering=False, debug=False, num_devices=NCORES)

    xt = nc.dram_tensor("xt", [D, NI], BF16, kind="ExternalInput").ap()
    wqt = nc.dram_tensor("wqt", [D, D], BF16, kind="ExternalInput").ap()
    wkt = nc.dram_tensor("wkt", [D, D], BF16, kind="ExternalInput").ap()
    wvt = nc.dram_tensor("wvt", [D, D], BF16, kind="ExternalInput").ap()
    wot = nc.dram_tensor("wot", [D, D], BF16, kind="ExternalInput").ap()
    bo = nc.dram_tensor("bo", [1, D], F32, kind="ExternalInput").ap()
    out = nc.dram_tensor("out", [NI, D], BF16, kind="ExternalOutput").ap()

    VROW = H * (DH + 1)  # 1040
    ccv_in = nc.dram_tensor("ccv_in", [NI, VROW], BF16, kind="Internal").ap()
    ccv_out = nc.dram_tensor("ccv_out", [NJ, VROW], BF16, kind="Internal").ap()
    cck_in = nc.dram_tensor("cck_in", [D, NI], BF16, kind="Internal").ap()
    cck_out = nc.dram_tensor("cck_out", [2 * D, NI], BF16, kind="Internal").ap()
    groups = [[2 * b, 2 * b + 1] for b in range(4)]

    with tile.TileContext(nc) as tc, ExitStack() as octx:
        # PSUM: psA (scores) 2x2 banks, psB (proj/Wo) 2x1, psC (AV) 2x1 = 8
        psA = octx.enter_context(tc.tile_pool(name="psA", bufs=2, space="PSUM"))
        psB = octx.enter_context(tc.tile_pool(name="psB", bufs=2, space="PSUM"))
        psC = octx.enter_context(tc.tile_pool(name="psC", bufs=2, space="PSUM"))

        # persistent SBUF
        kt_pool = octx.enter_context(tc.tile_pool(name="ktp", bufs=1))
        qt_pool = octx.enter_context(tc.tile_pool(name="qtp", bufs=1))
        v_pool = octx.enter_context(tc.tile_pool(name="vp", bufs=1))
        KT = [kt_pool.tile([P, NJ], BF16, tag=f"kt{i}", name=f"kt{i}") for i in range(NDB)]
        QT = [qt_pool.tile([P, NI], BF16, tag=f"qt{i}", name=f"qt{i}") for i in range(NDB)]
        vall = v_pool.tile([P, NJ // P, H, DH + 1], BF16, tag="vall", name="vall")
        V = [vall[:, j] for j in range(NJ // P)]

        ctx_pool = octx.enter_context(tc.tile_pool(name="ctxp", bufs=1, side="right"))
        CTX = [ctx_pool.tile([P, NI], BF16, tag=f"ctx{t}", name=f"ctx{t}") for t in range(NDB)]

        xtp = octx.enter_context(tc.tile_pool(name="xtp", bufs=1))
        XT = [xtp.tile([P, NI], BF16, tag=f"xt{c}", name=f"xt{c}") for c in range(NCC)]
        for c in range(NCC):
            nc.sync.dma_start(XT[c][:], xt[c * P : (c + 1) * P, :])

        # ---------------- phase V: own-half V projection + exchange ---------
        with (
            tc.tile_pool(name="wv", bufs=1) as wvp,
            tc.tile_pool(name="vown", bufs=1) as vop,
        ):
            WV = [wvp.tile([P, D], BF16, tag=f"wv{c}", name=f"wv{c}") for c in range(NCC)]
            for c in range(NCC):
                nc.sync.dma_start(WV[c][:], wvt[c * P : (c + 1) * P, :])
            vown = vop.tile([P, NI // P, H, DH + 1], BF16, tag="vown", name="vown")
            nc.vector.memset(vown[:, :, :, DH : DH + 1], 1.0)
            for j in range(NI // P):
                for vh in range(2):
                    ps = psB.tile([P, 512], F32, tag="pj", name="pj")
                    for c in range(NCC):
                        nc.tensor.matmul(
                            ps[:],
                            XT[c][:, j * P : (j + 1) * P],
                            WV[c][:, vh * 512 : (vh + 1) * 512],
                            start=(c == 0),
                            stop=(c == NCC - 1),
                        )
                    nc.vector.tensor_copy(
                        vown[:, j, vh * 8 : (vh + 1) * 8, 0:DH],
                        ps[:].rearrange("p (h d) -> p h d", h=8),
                    )
            for j in range(NI // P):
                nc.sync.dma_start(ccv_in[j * P : (j + 1) * P, :], vown[:, j])
        nc.gpsimd.collective_compute(
            "AllGather",
            mybir.AluOpType.bypass,
            ins=[ccv_in],
            outs=[ccv_out],
            replica_groups=groups,
        )
        for j in range(NJ // P):
            nc.sync.dma_start(V[j][:], ccv_out[j * P : (j + 1) * P, :])

        # ---------------- phase K: own-half KT projection + exchange --------
        with (
            tc.tile_pool(name="wk", bufs=1) as wkp,
            tc.tile_pool(name="kown", bufs=1) as kop,
        ):
            WK = [wkp.tile([P, D], BF16, tag=f"wk{c}", name=f"wk{c}") for c in range(NCC)]
            for c in range(NCC):
                nc.sync.dma_start(WK[c][:], wkt[c * P : (c + 1) * P, :])
            KO = [kop.tile([P, NI], BF16, tag=f"ko{t}", name=f"ko{t}") for t in range(NDB)]
            for db in range(NDB):
                for ib in range(NI // 512):
                    ps = psB.tile([P, 512], F32, tag="pj", name="pj")
                    for c in range(NCC):
                        nc.tensor.matmul(
                            ps[:],
                            WK[c][:, db * P : (db + 1) * P],
                            XT[c][:, ib * 512 : (ib + 1) * 512],
                            start=(c == 0),
                            stop=(c == NCC - 1),
                        )
                    nc.vector.tensor_copy(KO[db][:, ib * 512 : (ib + 1) * 512], ps[:])
                nc.sync.dma_start(cck_in[db * P : (db + 1) * P, :], KO[db][:])
        nc.gpsimd.collective_compute(
            "AllGather",
            mybir.AluOpType.bypass,
            ins=[cck_in],
            outs=[cck_out],
            replica_groups=groups,
        )
        for t in range(NDB):
            nc.sync.dma_start(KT[t][:, 0:NI], cck_out[t * P : (t + 1) * P, :])
            nc.sync.dma_start(KT[t][:, NI:NJ], cck_out[D + t * P : D + (t + 1) * P, :])

        # ---------------- phase Q ------------------------------------------
        with tc.tile_pool(name="wq", bufs=1) as wqp:
            WQ = [wqp.tile([P, D], BF16, tag=f"wq{c}", name=f"wq{c}") for c in range(NCC)]
            for c in range(NCC):
                nc.sync.dma_start(WQ[c][:], wqt[c * P : (c + 1) * P, :])
            for db in range(NDB):
                for ib in range(NI // 512):
                    ps = psB.tile([P, 512], F32, tag="pj", name="pj")
                    for c in range(NCC):
                        nc.tensor.matmul(
                            ps[:],
                            WQ[c][:, db * P : (db + 1) * P],
                            XT[c][:, ib * 512 : (ib + 1) * 512],
                            start=(c == 0),
                            stop=(c == NCC - 1),
                        )
                    nc.vector.tensor_copy(QT[db][:, ib * 512 : (ib + 1) * 512], ps[:])

        # Wo weights + bias prefetch (used at the end of each ib pass)
        wop = octx.enter_context(tc.tile_pool(name="wo", bufs=1))
        bip = octx.enter_context(tc.tile_pool(name="bias", bufs=1))
        osp = octx.enter_context(tc.tile_pool(name="os", bufs=6))
        WO = [wop.tile([P, D], BF16, tag=f"wo{f}", name=f"wo{f}") for f in range(NCC)]
        for f in range(NCC):
            nc.sync.dma_start(WO[f][:], wot[f * P : (f + 1) * P, :])
        BIAS = bip.tile([P, D], F32, name="BIAS")
        nc.gpsimd.dma_start(BIAS[:], bo.to_broadcast([P, D]))

        # ---------------- attention: ib-outer, head-pair inner --------------
        esp = octx.enter_context(tc.tile_pool(name="es", bufs=12))
        stp = octx.enter_context(tc.tile_pool(name="stg", bufs=10))
        recp = octx.enter_context(tc.tile_pool(name="rec", bufs=6))

        def emit_exp(es, sp, k):
            kind = EXP_POLICY[k % len(EXP_POLICY)]
            if kind == "a":
                nc.scalar.activation(es[:], sp[:], EXP, scale=SCALE)
            elif kind == "v":
                nc.vector.tensor_scalar(
                    es[:].bitcast(I16), sp[:], SCALE * EXP_C1, EXP_C2,
                    mybir.AluOpType.mult, mybir.AluOpType.add,
                )
            else:
                nc.gpsimd.tensor_scalar(
                    es[:].bitcast(I16), sp[:], SCALE * EXP_C1, EXP_C2,
                    mybir.AluOpType.mult, mybir.AluOpType.add,
                )

        for ib in range(NI // 512):
            islc = slice(ib * 512, (ib + 1) * 512)
            for db in range(NDB):
                t = db
                stgs = {}
                for hh in range(2):
                    h = 2 * db + hh
                    dp = hh * DH
                    es_list = []
                    for pr in range(NJ // 256):
                        sp = psA.tile([P, 1024], F32, tag="sp", name="sp")
                        for half2 in range(2):
                            j = pr * 2 + half2
                            nc.tensor.matmul(
                                sp[:, half2 * 512 : (half2 + 1) * 512],
                                KT[t][dp : dp + DH, j * P : (j + 1) * P],
                                QT[t][dp : dp + DH, islc],
                                start=True,
                                stop=True,
                            )
                        es = esp.tile([P, 1024], BF16, tag="es", name="es")
                        emit_exp(es, sp, pr)
                        es_list.append(es)
                    # AV: es stationary, V|1 moving -> out [queries, dh+1];
                    # per-partition divide by the ones-column sum normalizes.
                    for q in range(4):
                        ctp = psC.tile([P, DH + 1], F32, tag="ct", name="ct")
                        for j in range(NJ // P):
                            nc.tensor.matmul(
                                ctp[:],
                                es_list[j // 2][
                                    :,
                                    (j % 2) * 512 + q * P : (j % 2) * 512 + (q + 1) * P,
                                ],
                                V[j][:, h, :],
                                start=(j == 0),
                                stop=(j == NJ // P - 1),
                            )
                        rec = recp.tile([P, 1], F32, tag="rec", name="rec")
                        nc.vector.reciprocal(rec[:], ctp[:, DH : DH + 1])
                        if hh == 0:
                            stgs[q] = stp.tile([P, 2 * DH], BF16, tag="st", name="st")
                        stg = stgs[q]
                        nc.vector.tensor_scalar_mul(
                            stg[:, dp : dp + DH], ctp[:, 0:DH], rec[:]
                        )
                        if hh == 1:
                            nc.sync.dma_start_transpose(
                                CTX[t][:, ib * 512 + q * P : ib * 512 + (q + 1) * P],
                                stg[:],
                            )
            # Wo for this ib's four 128-query blocks
            for ib8 in range(ib * 4, ib * 4 + 4):
                for eb in range(2):
                    ps = psB.tile([P, 512], F32, tag="pj", name="pj")
                    for f in range(NCC):
                        nc.tensor.matmul(
                            ps[:],
                            CTX[f][:, ib8 * P : (ib8 + 1) * P],
                            WO[f][:, eb * 512 : (eb + 1) * 512],
                            start=(f == 0),
                            stop=(f == NCC - 1),
                        )
                    ostage = osp.tile([P, 512], BF16, tag="os", name="os")
                    nc.vector.tensor_add(
                        ostage[:], ps[:], BIAS[:, eb * 512 : (eb + 1) * 512]
                    )
                    nc.sync.dma_start(
                        out[ib8 * P : (ib8 + 1) * P, eb * 512 : (eb + 1) * 512],
                        ostage[:],
                    )

    nc.compile()
    return nc


_NC = None


def _get_nc():
    global _NC
    if _NC is None:
        _NC = _build()
    return _NC


def _make_in_maps(x, Wq, Wk, Wv, Wo, bo):
    import ml_dtypes

    bf16 = ml_dtypes.bfloat16
    wqt = np.ascontiguousarray(Wq.T).astype(bf16)
    wkt = np.ascontiguousarray(Wk.T).astype(bf16)
    wvt = np.ascontiguousarray(Wv.T).astype(bf16)
    wot = np.ascontiguousarray(Wo.T).astype(bf16)
    bo2 = np.ascontiguousarray(bo.reshape(1, D)).astype(np.float32)
    in_maps = []
    for c in range(NCORES):
        b, s = c // 2, c % 2
        xt = np.ascontiguousarray(x[b, s * NI : (s + 1) * NI, :].T).astype(bf16)
        in_maps.append(
            {"xt": xt, "wqt": wqt, "wkt": wkt, "wvt": wvt, "wot": wot, "bo": bo2}
        )
    return in_maps


def _run(x, Wq, Wk, Wv, Wo, bo, **spmd_kwargs):
    nc = _get_nc()
    in_maps = _make_in_maps(x, Wq, Wk, Wv, Wo, bo)
    res = run_bass_kernel_spmd(nc, in_maps, list(range(NCORES)), **spmd_kwargs)
    outs = [np.asarray(res.results[c]["out"]) for c in range(NCORES)]
    full = np.concatenate(outs, axis=0).reshape(4, 2048, D).astype(np.float32)
    return full, res


def kernel(x, Wq, Wk, Wv, Wo, bo):
    full, _ = _run(
        np.asarray(x), np.asarray(Wq), np.asarray(Wk), np.asarray(Wv),
        np.asarray(Wo), np.asarray(bo),
    )
    return full


# revision 17
# speedup vs baseline: 1.6140x; 1.6140x over previous
"""Multi-head attention (b=4, n=2048, d=1024, h=16, dh=64) on 8 TRN2 NeuronCores.

Sharding: batch x sequence-half per core (core c handles batch b=c//2, query
rows s=(c%2)*1024 .. +1024). Each core recomputes K/V for its whole batch
locally (no collectives), computes flash-style attention for its 1024 query
rows over all 16 heads, applies the output projection, and writes a disjoint
1024-row slice of the flattened output.

Host-side layout choices (free transposes/permutes in numpy):
  xtkv [d, 2048] = concat(x[b, my_half].T, x[b, other_half].T) -- the core's
      own query rows are ALWAYS columns 0:1024, so the same SPMD graph works
      on every core, and key order permutation is softmax-invariant.
  wqt/wkt/wvt/wot = W.T (contraction dim first), bo as [1, d].

Softmax exp is the ScalarE bottleneck (1 elem/cycle/lane), so the per-group
exp tiles are striped across ScalarE (exact), DVE and GPSIMD (both via the
bf16 Schraudolph bit trick: bf16_bits(exp(s*SCALE)) ~= rint(s*C1' + C2) as a
single tensor_scalar into an int16 view). The AV consumption of each
(head, ib) group is deferred by one group (software pipeline) so TensorE has
score work to do while the previous group's exp tiles drain.
"""

import sys

sys.path.insert(0, "/opt/trn_rl_repo")

from contextlib import ExitStack

import numpy as np

import concourse.bass as bass
import concourse.tile as tile
from concourse import bacc, mybir
from concourse.bass_utils import run_bass_kernel_spmd

F32 = mybir.dt.float32
BF16 = mybir.dt.bfloat16
I16 = mybir.dt.int16
EXP = mybir.ActivationFunctionType.Exp

P = 128
D = 1024  # model dim
NI = 1024  # query rows per core
NJ = 2048  # key rows per core (full batch)
H = 16  # heads
DH = 64  # head dim
SCALE = DH**-0.5  # 0.125
NCORES = 8

NCC = D // P  # 8 contraction chunks
NDB = D // P  # 8 feature blocks

# Schraudolph bf16 exp constants (tuned in micro_exp.py; rint conversion).
EXP_C1 = float(128.0 * np.log2(np.e))
EXP_C2 = 16249.0

# Per-group exp engine stripe: 'a' = ScalarE exact, 'v' = DVE trick.
# (GPSIMD cannot read PSUM, so only ScalarE/DVE can consume score tiles.)
# Striped so neither engine gets back-to-back tiles beyond its drain rate.
EXP_POLICY = "aaaaaaaa"


def _build():
    nc = bacc.Bacc("TRN2", target_bir_lowering=False, debug=False, num_devices=NCORES)

    xtkv = nc.dram_tensor("xtkv", [D, NJ], BF16, kind="ExternalInput").ap()
    wqt = nc.dram_tensor("wqt", [D, D], BF16, kind="ExternalInput").ap()
    wkt = nc.dram_tensor("wkt", [D, D], BF16, kind="ExternalInput").ap()
    wvt = nc.dram_tensor("wvt", [D, D], BF16, kind="ExternalInput").ap()
    wot = nc.dram_tensor("wot", [D, D], BF16, kind="ExternalInput").ap()
    bo = nc.dram_tensor("bo", [1, D], F32, kind="ExternalInput").ap()
    out = nc.dram_tensor("out", [NI, D], BF16, kind="ExternalOutput").ap()

    with tile.TileContext(nc) as tc, ExitStack() as octx:
        # kernel-wide PSUM pools: 4 + 2 + 2 = 8 banks
        psA = octx.enter_context(tc.tile_pool(name="psA", bufs=2, space="PSUM"))
        psB = octx.enter_context(tc.tile_pool(name="psB", bufs=2, space="PSUM"))
        psC = octx.enter_context(tc.tile_pool(name="psC", bufs=2, space="PSUM"))

        kt_pool = octx.enter_context(tc.tile_pool(name="ktp", bufs=1))
        qt_pool = octx.enter_context(tc.tile_pool(name="qtp", bufs=1))
        v_pool = octx.enter_context(tc.tile_pool(name="vp", bufs=1))
        KT = [kt_pool.tile([P, NJ], BF16, tag=f"kt{i}", name=f"kt{i}") for i in range(NDB)]
        QT = [qt_pool.tile([P, NI], BF16, tag=f"qt{i}", name=f"qt{i}") for i in range(NDB)]
        vall = v_pool.tile([P, NJ // P, H, DH + 1], BF16, tag="vall", name="vall")
        V = [vall[:, j] for j in range(NJ // P)]

        # attention pools first: their SBUF must not alias the projection pools
        ctx_pool = octx.enter_context(tc.tile_pool(name="ctxp", bufs=1, side="right"))
        CTX = [ctx_pool.tile([P, NI], BF16, tag=f"ctx{t}", name=f"ctx{t}") for t in range(NDB)]
        esp = octx.enter_context(tc.tile_pool(name="es", bufs=16))
        recp = octx.enter_context(tc.tile_pool(name="rec", bufs=6))
        stp = octx.enter_context(tc.tile_pool(name="stg", bufs=10))

        # ---------------- phase Q (bf16); XQ reused by K; WK/XKB prefetched ----
        xqp = octx.enter_context(tc.tile_pool(name="xq", bufs=1))
        XQ = [xqp.tile([P, NI], BF16, tag=f"xq{c}", name=f"xq{c}") for c in range(NCC)]
        wkp = octx.enter_context(tc.tile_pool(name="wk", bufs=1))
        WK = [wkp.tile([P, D], BF16, tag=f"wk{c}", name=f"wk{c}") for c in range(NCC)]
        with tc.tile_pool(name="wq", bufs=1) as wqp:
            WQ = [wqp.tile([P, D], BF16, tag=f"wq{c}", name=f"wq{c}") for c in range(NCC)]
            for c in range(NCC):
                nc.sync.dma_start(XQ[c][:], xtkv[c * P : (c + 1) * P, 0:NI])
                nc.sync.dma_start(WQ[c][:], wqt[c * P : (c + 1) * P, :])
            for c in range(NCC):
                nc.sync.dma_start(WK[c][:], wkt[c * P : (c + 1) * P, :])
            for db in range(NDB):
                for ib in range(NI // 512):
                    ps = psB.tile([P, 512], F32, tag="pj", name="pj")
                    for c in range(NCC):
                        nc.tensor.matmul(
                            ps[:],
                            WQ[c][:, db * P : (db + 1) * P],
                            XQ[c][:, ib * 512 : (ib + 1) * 512],
                            start=(c == 0),
                            stop=(c == NCC - 1),
                        )
                    nc.vector.tensor_copy(QT[db][:, ib * 512 : (ib + 1) * 512], ps[:])

        # bias prefetch; WO reuses the WK pool buffers once K proj finishes
        bip = octx.enter_context(tc.tile_pool(name="bias", bufs=1))
        osp = octx.enter_context(tc.tile_pool(name="os", bufs=5))
        BIAS = bip.tile([P, D], F32, name="BIAS")
        nc.gpsimd.dma_start(BIAS[:], bo.to_broadcast([P, D]))
        WO = [None] * NCC

        def emit_exp(es, sp, k):
            kind = EXP_POLICY[k % len(EXP_POLICY)]
            if kind == "a":
                nc.scalar.activation(es[:], sp[:], EXP, scale=SCALE)
            else:
                nc.vector.tensor_scalar(
                    es[:].bitcast(I16), sp[:], SCALE * EXP_C1, EXP_C2,
                    mybir.AluOpType.mult, mybir.AluOpType.add,
                )

        # -------- fused phase K + attention: per db, project KT[db], emit the
        # scores+exp for its two heads; the AV of each (h, ib) group is
        # deferred one group so exp tiles drain behind score/proj matmuls ----
        stgs = {}  # (db, q) -> staging tile shared by the hh pair
        DQ = DH + 1  # 65; 4 q-slices side by side in one psum tile

        def emit_av_chunk(g, j0, j1):
            """AV matmuls for key blocks [j0, j1) of group g into g's shared
            psum tile (4 q-slices of [128, 65])."""
            db, ib, hh, es_list, ctp = g
            h = 2 * db + hh
            for j in range(j0, j1):
                for q in range(4):
                    nc.tensor.matmul(
                        ctp[:, q * DQ : (q + 1) * DQ],
                        es_list[j // 2][
                            :,
                            (j % 2) * 512 + q * P : (j % 2) * 512 + (q + 1) * P,
                        ],
                        V[j][:, h, :],
                        # start=True clears the whole bank's has_written bits,
                        # so only the tile's first matmul may set it; later
                        # first-writes per q-slice overwrite via has_written.
                        start=(j == 0 and q == 0),
                        stop=(j == NJ // P - 1),
                    )

        def emit_av_epilogue(g):
            db, ib, hh, es_list, ctp = g
            t = db
            dp = hh * DH
            for q in range(4):
                rec = recp.tile([P, 1], F32, tag="rec", name="rec")
                nc.vector.reciprocal(rec[:], ctp[:, q * DQ + DH : q * DQ + DH + 1])
                if hh == 0:
                    stgs[(db, q)] = stp.tile([P, 2 * DH], BF16, tag="st", name="st")
                stg = stgs[(db, q)]
                nc.vector.tensor_scalar_mul(
                    stg[:, dp : dp + DH], ctp[:, q * DQ : q * DQ + DH], rec[:]
                )
                if hh == 1:
                    nc.sync.dma_start_transpose(
                        CTX[t][:, ib * 512 + q * P : ib * 512 + (q + 1) * P],
                        stg[:],
                    )

        with (
            tc.tile_pool(name="xkb", bufs=1) as xkbp,
            tc.tile_pool(name="wvh", bufs=1) as wvhp,
        ):
            XKB = [xkbp.tile([P, NI], BF16, tag=f"xkb{c}", name=f"xkb{c}") for c in range(NCC)]
            XKA = XQ
            for c in range(NCC):
                nc.sync.dma_start(XKB[c][:], xtkv[c * P : (c + 1) * P, NI:NJ])
            for j in range(NJ // P):
                nc.vector.memset(V[j][:, :, DH : DH + 1], 1.0)

            def v_halfpass_jg(vh, jg):
                """Project V head-half vh for key group jg (4 j-blocks); x is
                sliced straight out of the resident XKA/XKB tiles."""
                for j4 in range(4):
                    j = jg * 4 + j4
                    xh = XKA if j < 8 else XKB
                    jloc = j % 8
                    ps = psB.tile([P, 512], F32, tag="pj", name="pj")
                    for c in range(NCC):
                        nc.tensor.matmul(
                            ps[:],
                            xh[c][:, jloc * P : (jloc + 1) * P],
                            WVH[c][:],
                            start=(c == 0),
                            stop=(c == NCC - 1),
                        )
                    nc.vector.tensor_copy(
                        V[j][:, vh * 8 : (vh + 1) * 8, 0:DH],
                        ps[:].rearrange("p (h d) -> p h d", h=8),
                    )

            WVH = [wvhp.tile([P, 512], BF16, tag=f"wvh{c}", name=f"wvh{c}") for c in range(NCC)]
            # V head-half 0 (heads 0-7): needed from db=0
            for c in range(NCC):
                nc.sync.dma_start(WVH[c][:], wvt[c * P : (c + 1) * P, 0:512])
            for jg in range(NJ // 512):
                v_halfpass_jg(0, jg)

            prev = None
            for db in range(NDB):
                # V head-half 1 (heads 8-15): one key group per db in 1..4
                if db == 1:
                    WVH = [
                        wvhp.tile([P, 512], BF16, tag=f"wvh{c}", name=f"wvh{c}2")
                        for c in range(NCC)
                    ]
                    for c in range(NCC):
                        nc.sync.dma_start(WVH[c][:], wvt[c * P : (c + 1) * P, 512:1024])
                if 1 <= db <= 4:
                    v_halfpass_jg(1, db - 1)
                # K projection for this db
                for jb in range(NJ // 512):
                    half = XKA if jb < 2 else XKB
                    cslc = slice((jb % 2) * 512, (jb % 2) * 512 + 512)
                    ps = psB.tile([P, 512], F32, tag="pj", name="pj")
                    for c in range(NCC):
                        nc.tensor.matmul(
                            ps[:],
                            WK[c][:, db * P : (db + 1) * P],
                            half[c][:, cslc],
                            start=(c == 0),
                            stop=(c == NCC - 1),
                        )
                    nc.vector.tensor_copy(KT[db][:, jb * 512 : (jb + 1) * 512], ps[:])
                if db == NDB - 1:
                    # WK buffers are dead now; stage the Wo weights in them so
                    # the loads overlap the last head-pair's attention work.
                    for f in range(NCC):
                        WO[f] = wkp.tile([P, D], BF16, tag=f"wk{f}", name=f"wo{f}")
                        nc.sync.dma_start(WO[f][:], wot[f * P : (f + 1) * P, :])
                # scores + exp for the two heads in KT[db]; AV lags one group
                t = db
                for ib in range(NI // 512):
                    islc = slice(ib * 512, (ib + 1) * 512)
                    for hh in range(2):
                        dp = hh * DH
                        es_list = []
                        ctp = psC.tile([P, 4 * DQ], F32, tag="ct", name="ct")
                        g = (db, ib, hh, es_list, ctp)
                        for pr in range(NJ // 256):
                            sp = psA.tile([P, 1024], F32, tag="sp", name="sp")
                            for half2 in range(2):
                                j = pr * 2 + half2
                                nc.tensor.matmul(
                                    sp[:, half2 * 512 : (half2 + 1) * 512],
                                    KT[t][dp : dp + DH, j * P : (j + 1) * P],
                                    QT[t][dp : dp + DH, islc],
                                    start=True,
                                    stop=True,
                                )
                            es = esp.tile([P, 1024], BF16, tag="es", name="es")
                            emit_exp(es, sp, pr)
                            es_list.append(es)
                            if prev is not None:
                                # interleave the previous group's AV so TensorE
                                # has work while this group's exp tiles drain
                                emit_av_chunk(prev, 2 * pr, 2 * pr + 2)
                                if pr == NJ // 256 - 1:
                                    emit_av_epilogue(prev)
                        prev = g
            emit_av_chunk(prev, 0, NJ // P)
            emit_av_epilogue(prev)

        # ---------------- phase Wo: out = CTX.T @ WoT + bo ----------------------
        for ib8 in range(NI // P):
            for eb in range(2):
                ps = psB.tile([P, 512], F32, tag="pj", name="pj")
                for f in range(NCC):
                    nc.tensor.matmul(
                        ps[:],
                        CTX[f][:, ib8 * P : (ib8 + 1) * P],
                        WO[f][:, eb * 512 : (eb + 1) * 512],
                        start=(f == 0),
                        stop=(f == NCC - 1),
                    )
                ostage = osp.tile([P, 512], BF16, tag="os", name="os")
                nc.vector.tensor_add(
                    ostage[:], ps[:], BIAS[:, eb * 512 : (eb + 1) * 512]
                )
                nc.sync.dma_start(
                    out[ib8 * P : (ib8 + 1) * P, eb * 512 : (eb + 1) * 512],
                    ostage[:],
                )

    nc.compile()
    return nc


_NC = None


def _get_nc():
    global _NC
    if _NC is None:
        _NC = _build()
    return _NC


def _make_in_maps(x, Wq, Wk, Wv, Wo, bo):
    import ml_dtypes

    bf16 = ml_dtypes.bfloat16
    wqt = np.ascontiguousarray(Wq.T).astype(bf16)
    wkt = np.ascontiguousarray(Wk.T).astype(bf16)
    wvt = np.ascontiguousarray(Wv.T).astype(bf16)
    wot = np.ascontiguousarray(Wo.T).astype(bf16)
    bo2 = np.ascontiguousarray(bo.reshape(1, D)).astype(np.float32)
    in_maps = []
    for c in range(NCORES):
        b, s = c // 2, c % 2
        mine = x[b, s * NI : (s + 1) * NI, :].T
        other = x[b, (1 - s) * NI : (2 - s) * NI, :].T
        xtkv = np.ascontiguousarray(np.concatenate([mine, other], axis=1)).astype(bf16)
        in_maps.append(
            {"xtkv": xtkv, "wqt": wqt, "wkt": wkt, "wvt": wvt, "wot": wot, "bo": bo2}
        )
    return in_maps


def _run(x, Wq, Wk, Wv, Wo, bo, **spmd_kwargs):
    nc = _get_nc()
    in_maps = _make_in_maps(x, Wq, Wk, Wv, Wo, bo)
    res = run_bass_kernel_spmd(nc, in_maps, list(range(NCORES)), **spmd_kwargs)
    outs = [np.asarray(res.results[c]["out"]) for c in range(NCORES)]
    full = np.concatenate(outs, axis=0).reshape(4, 2048, D).astype(np.float32)
    return full, res


def kernel(x, Wq, Wk, Wv, Wo, bo):
    full, _ = _run(
        np.asarray(x), np.asarray(Wq), np.asarray(Wk), np.asarray(Wv),
        np.asarray(Wo), np.asarray(bo),
    )
    return full


# revision 18
# speedup vs baseline: 1.6473x; 1.0206x over previous
"""Multi-head attention (b=4, n=2048, d=1024, h=16, dh=64) on 8 TRN2 NeuronCores.

Sharding: batch x sequence-half per core (core c handles batch b=c//2, query
rows s=(c%2)*1024 .. +1024). Each core recomputes K/V for its whole batch
locally (no collectives), computes flash-style attention for its 1024 query
rows over all 16 heads, applies the output projection, and writes a disjoint
1024-row slice of the flattened output.

Host-side layout choices (free transposes/permutes in numpy):
  xtkv [d, 2048] = concat(x[b, my_half].T, x[b, other_half].T) -- the core's
      own query rows are ALWAYS columns 0:1024, so the same SPMD graph works
      on every core, and key order permutation is softmax-invariant.
  wqt/wkt/wvt/wot = W.T (contraction dim first), bo as [1, d].

Softmax exp is the ScalarE bottleneck (1 elem/cycle/lane), so the per-group
exp tiles are striped across ScalarE (exact), DVE and GPSIMD (both via the
bf16 Schraudolph bit trick: bf16_bits(exp(s*SCALE)) ~= rint(s*C1' + C2) as a
single tensor_scalar into an int16 view). The AV consumption of each
(head, ib) group is deferred by one group (software pipeline) so TensorE has
score work to do while the previous group's exp tiles drain.
"""

import sys

sys.path.insert(0, "/opt/trn_rl_repo")

from contextlib import ExitStack

import numpy as np

import concourse.bass as bass
import concourse.tile as tile
from concourse import bacc, mybir
from concourse.bass_utils import run_bass_kernel_spmd

F32 = mybir.dt.float32
BF16 = mybir.dt.bfloat16
I16 = mybir.dt.int16
EXP = mybir.ActivationFunctionType.Exp

P = 128
D = 1024  # model dim
NI = 1024  # query rows per core
NJ = 2048  # key rows per core (full batch)
H = 16  # heads
DH = 64  # head dim
SCALE = DH**-0.5  # 0.125
NCORES = 8

NCC = D // P  # 8 contraction chunks
NDB = D // P  # 8 feature blocks

# Schraudolph bf16 exp constants (tuned in micro_exp.py; rint conversion).
EXP_C1 = float(128.0 * np.log2(np.e))
EXP_C2 = 16249.0

# Per-group exp engine stripe: 'a' = ScalarE exact, 'v' = DVE trick.
# (GPSIMD cannot read PSUM, so only ScalarE/DVE can consume score tiles.)
# Striped so neither engine gets back-to-back tiles beyond its drain rate.
EXP_POLICY = "avaavaav"


def _build():
    nc = bacc.Bacc("TRN2", target_bir_lowering=False, debug=False, num_devices=NCORES)

    xtkv = nc.dram_tensor("xtkv", [D, NJ], BF16, kind="ExternalInput").ap()
    wqt = nc.dram_tensor("wqt", [D, D], BF16, kind="ExternalInput").ap()
    wkt = nc.dram_tensor("wkt", [D, D], BF16, kind="ExternalInput").ap()
    wvt = nc.dram_tensor("wvt", [D, D], BF16, kind="ExternalInput").ap()
    wot = nc.dram_tensor("wot", [D, D], BF16, kind="ExternalInput").ap()
    bo = nc.dram_tensor("bo", [1, D], F32, kind="ExternalInput").ap()
    out = nc.dram_tensor("out", [NI, D], BF16, kind="ExternalOutput").ap()

    with tile.TileContext(nc) as tc, ExitStack() as octx:
        # kernel-wide PSUM pools: 4 + 2 + 2 = 8 banks
        psA = octx.enter_context(tc.tile_pool(name="psA", bufs=2, space="PSUM"))
        psB = octx.enter_context(tc.tile_pool(name="psB", bufs=2, space="PSUM"))
        psC = octx.enter_context(tc.tile_pool(name="psC", bufs=2, space="PSUM"))

        kt_pool = octx.enter_context(tc.tile_pool(name="ktp", bufs=1))
        qt_pool = octx.enter_context(tc.tile_pool(name="qtp", bufs=1))
        v_pool = octx.enter_context(tc.tile_pool(name="vp", bufs=1))
        KT = [kt_pool.tile([P, NJ], BF16, tag=f"kt{i}", name=f"kt{i}") for i in range(NDB)]
        QT = [qt_pool.tile([P, NI], BF16, tag=f"qt{i}", name=f"qt{i}") for i in range(NDB)]
        vall = v_pool.tile([P, NJ // P, H, DH + 1], BF16, tag="vall", name="vall")
        V = [vall[:, j] for j in range(NJ // P)]

        # attention pools first: their SBUF must not alias the projection pools
        ctx_pool = octx.enter_context(tc.tile_pool(name="ctxp", bufs=1, side="right"))
        CTX = [ctx_pool.tile([P, NI], BF16, tag=f"ctx{t}", name=f"ctx{t}") for t in range(NDB)]
        esp = octx.enter_context(tc.tile_pool(name="es", bufs=16))
        recp = octx.enter_context(tc.tile_pool(name="rec", bufs=6))
        stp = octx.enter_context(tc.tile_pool(name="stg", bufs=10))

        # ---------------- phase Q (bf16); XQ reused by K; WK/XKB prefetched ----
        xqp = octx.enter_context(tc.tile_pool(name="xq", bufs=1))
        XQ = [xqp.tile([P, NI], BF16, tag=f"xq{c}", name=f"xq{c}") for c in range(NCC)]
        wkp = octx.enter_context(tc.tile_pool(name="wk", bufs=1))
        WK = [wkp.tile([P, D], BF16, tag=f"wk{c}", name=f"wk{c}") for c in range(NCC)]
        with tc.tile_pool(name="wq", bufs=1) as wqp:
            WQ = [wqp.tile([P, D], BF16, tag=f"wq{c}", name=f"wq{c}") for c in range(NCC)]
            for c in range(NCC):
                nc.sync.dma_start(XQ[c][:], xtkv[c * P : (c + 1) * P, 0:NI])
                nc.sync.dma_start(WQ[c][:], wqt[c * P : (c + 1) * P, :])
            for c in range(NCC):
                nc.sync.dma_start(WK[c][:], wkt[c * P : (c + 1) * P, :])
            for db in range(NDB):
                for ib in range(NI // 512):
                    ps = psB.tile([P, 512], F32, tag="pj", name="pj")
                    for c in range(NCC):
                        nc.tensor.matmul(
                            ps[:],
                            WQ[c][:, db * P : (db + 1) * P],
                            XQ[c][:, ib * 512 : (ib + 1) * 512],
                            start=(c == 0),
                            stop=(c == NCC - 1),
                        )
                    nc.vector.tensor_copy(QT[db][:, ib * 512 : (ib + 1) * 512], ps[:])

        # bias prefetch; WO reuses the WK pool buffers once K proj finishes
        bip = octx.enter_context(tc.tile_pool(name="bias", bufs=1))
        osp = octx.enter_context(tc.tile_pool(name="os", bufs=5))
        BIAS = bip.tile([P, D], F32, name="BIAS")
        nc.gpsimd.dma_start(BIAS[:], bo.to_broadcast([P, D]))
        WO = [None] * NCC

        def emit_exp(es, sp, k):
            kind = EXP_POLICY[k % len(EXP_POLICY)]
            if kind == "a":
                nc.scalar.activation(es[:], sp[:], EXP, scale=SCALE)
            else:
                nc.vector.tensor_scalar(
                    es[:].bitcast(I16), sp[:], SCALE * EXP_C1, EXP_C2,
                    mybir.AluOpType.mult, mybir.AluOpType.add,
                )

        # -------- fused phase K + attention: per db, project KT[db], emit the
        # scores+exp for its two heads; the AV of each (h, ib) group is
        # deferred one group so exp tiles drain behind score/proj matmuls ----
        stgs = {}  # (db, q) -> staging tile shared by the hh pair
        DQ = DH + 1  # 65; 4 q-slices side by side in one psum tile

        def emit_av_chunk(g, j0, j1):
            """AV matmuls for key blocks [j0, j1) of group g into g's shared
            psum tile (4 q-slices of [128, 65])."""
            db, ib, hh, es_list, ctp = g
            h = 2 * db + hh
            for j in range(j0, j1):
                for q in range(4):
                    nc.tensor.matmul(
                        ctp[:, q * DQ : (q + 1) * DQ],
                        es_list[j // 2][
                            :,
                            (j % 2) * 512 + q * P : (j % 2) * 512 + (q + 1) * P,
                        ],
                        V[j][:, h, :],
                        # start=True clears the whole bank's has_written bits,
                        # so only the tile's first matmul may set it; later
                        # first-writes per q-slice overwrite via has_written.
                        start=(j == 0 and q == 0),
                        stop=(j == NJ // P - 1),
                    )

        def emit_av_epilogue(g):
            db, ib, hh, es_list, ctp = g
            t = db
            dp = hh * DH
            for q in range(4):
                rec = recp.tile([P, 1], F32, tag="rec", name="rec")
                nc.vector.reciprocal(rec[:], ctp[:, q * DQ + DH : q * DQ + DH + 1])
                if hh == 0:
                    stgs[(db, q)] = stp.tile([P, 2 * DH], BF16, tag="st", name="st")
                stg = stgs[(db, q)]
                nc.vector.tensor_scalar_mul(
                    stg[:, dp : dp + DH], ctp[:, q * DQ : q * DQ + DH], rec[:]
                )
                if hh == 1:
                    nc.sync.dma_start_transpose(
                        CTX[t][:, ib * 512 + q * P : ib * 512 + (q + 1) * P],
                        stg[:],
                    )

        with (
            tc.tile_pool(name="xkb", bufs=1) as xkbp,
            tc.tile_pool(name="wvh", bufs=1) as wvhp,
        ):
            XKB = [xkbp.tile([P, NI], BF16, tag=f"xkb{c}", name=f"xkb{c}") for c in range(NCC)]
            XKA = XQ
            for c in range(NCC):
                nc.sync.dma_start(XKB[c][:], xtkv[c * P : (c + 1) * P, NI:NJ])
            for j in range(NJ // P):
                nc.vector.memset(V[j][:, :, DH : DH + 1], 1.0)

            def v_halfpass_jg(vh, jg):
                """Project V head-half vh for key group jg (4 j-blocks); x is
                sliced straight out of the resident XKA/XKB tiles."""
                for j4 in range(4):
                    j = jg * 4 + j4
                    xh = XKA if j < 8 else XKB
                    jloc = j % 8
                    ps = psB.tile([P, 512], F32, tag="pj", name="pj")
                    for c in range(NCC):
                        nc.tensor.matmul(
                            ps[:],
                            xh[c][:, jloc * P : (jloc + 1) * P],
                            WVH[c][:],
                            start=(c == 0),
                            stop=(c == NCC - 1),
                        )
                    nc.vector.tensor_copy(
                        V[j][:, vh * 8 : (vh + 1) * 8, 0:DH],
                        ps[:].rearrange("p (h d) -> p h d", h=8),
                    )

            WVH = [wvhp.tile([P, 512], BF16, tag=f"wvh{c}", name=f"wvh{c}") for c in range(NCC)]
            # V head-half 0 (heads 0-7): needed from db=0
            for c in range(NCC):
                nc.sync.dma_start(WVH[c][:], wvt[c * P : (c + 1) * P, 0:512])
            for jg in range(NJ // 512):
                v_halfpass_jg(0, jg)

            prev = None
            for db in range(NDB):
                # V head-half 1 (heads 8-15): one key group per db in 1..4
                if db == 1:
                    WVH = [
                        wvhp.tile([P, 512], BF16, tag=f"wvh{c}", name=f"wvh{c}2")
                        for c in range(NCC)
                    ]
                    for c in range(NCC):
                        nc.sync.dma_start(WVH[c][:], wvt[c * P : (c + 1) * P, 512:1024])
                if 1 <= db <= 4:
                    v_halfpass_jg(1, db - 1)
                # K projection for this db
                for jb in range(NJ // 512):
                    half = XKA if jb < 2 else XKB
                    cslc = slice((jb % 2) * 512, (jb % 2) * 512 + 512)
                    ps = psB.tile([P, 512], F32, tag="pj", name="pj")
                    for c in range(NCC):
                        nc.tensor.matmul(
                            ps[:],
                            WK[c][:, db * P : (db + 1) * P],
                            half[c][:, cslc],
                            start=(c == 0),
                            stop=(c == NCC - 1),
                        )
                    nc.vector.tensor_copy(KT[db][:, jb * 512 : (jb + 1) * 512], ps[:])
                if db == NDB - 1:
                    # WK buffers are dead now; stage the Wo weights in them so
                    # the loads overlap the last head-pair's attention work.
                    for f in range(NCC):
                        WO[f] = wkp.tile([P, D], BF16, tag=f"wk{f}", name=f"wo{f}")
                        nc.sync.dma_start(WO[f][:], wot[f * P : (f + 1) * P, :])
                # scores + exp for the two heads in KT[db]; AV lags one group
                t = db
                for ib in range(NI // 512):
                    islc = slice(ib * 512, (ib + 1) * 512)
                    for hh in range(2):
                        dp = hh * DH
                        es_list = []
                        ctp = psC.tile([P, 4 * DQ], F32, tag="ct", name="ct")
                        g = (db, ib, hh, es_list, ctp)
                        for pr in range(NJ // 256):
                            sp = psA.tile([P, 1024], F32, tag="sp", name="sp")
                            for half2 in range(2):
                                j = pr * 2 + half2
                                nc.tensor.matmul(
                                    sp[:, half2 * 512 : (half2 + 1) * 512],
                                    KT[t][dp : dp + DH, j * P : (j + 1) * P],
                                    QT[t][dp : dp + DH, islc],
                                    start=True,
                                    stop=True,
                                )
                            es = esp.tile([P, 1024], BF16, tag="es", name="es")
                            emit_exp(es, sp, pr)
                            es_list.append(es)
                            if prev is not None:
                                # interleave the previous group's AV so TensorE
                                # has work while this group's exp tiles drain
                                emit_av_chunk(prev, 2 * pr, 2 * pr + 2)
                                if pr == NJ // 256 - 1:
                                    emit_av_epilogue(prev)
                        prev = g
            emit_av_chunk(prev, 0, NJ // P)
            emit_av_epilogue(prev)

        # ---------------- phase Wo: out = CTX.T @ WoT + bo ----------------------
        for ib8 in range(NI // P):
            for eb in range(2):
                ps = psB.tile([P, 512], F32, tag="pj", name="pj")
                for f in range(NCC):
                    nc.tensor.matmul(
                        ps[:],
                        CTX[f][:, ib8 * P : (ib8 + 1) * P],
                        WO[f][:, eb * 512 : (eb + 1) * 512],
                        start=(f == 0),
                        stop=(f == NCC - 1),
                    )
                ostage = osp.tile([P, 512], BF16, tag="os", name="os")
                nc.vector.tensor_add(
                    ostage[:], ps[:], BIAS[:, eb * 512 : (eb + 1) * 512]
                )
                nc.sync.dma_start(
                    out[ib8 * P : (ib8 + 1) * P, eb * 512 : (eb + 1) * 512],
                    ostage[:],
                )

    nc.compile()
    return nc


_NC = None


def _get_nc():
    global _NC
    if _NC is None:
        _NC = _build()
    return _NC


def _make_in_maps(x, Wq, Wk, Wv, Wo, bo):
    import ml_dtypes

    bf16 = ml_dtypes.bfloat16
    wqt = np.ascontiguousarray(Wq.T).astype(bf16)
    wkt = np.ascontiguousarray(Wk.T).astype(bf16)
    wvt = np.ascontiguousarray(Wv.T).astype(bf16)
    wot = np.ascontiguousarray(Wo.T).astype(bf16)
    bo2 = np.ascontiguousarray(bo.reshape(1, D)).astype(np.float32)
    in_maps = []
    for c in range(NCORES):
        b, s = c // 2, c % 2
        mine = x[b, s * NI : (s + 1) * NI, :].T
        other = x[b, (1 - s) * NI : (2 - s) * NI, :].T
        xtkv = np.ascontiguousarray(np.concatenate([mine, other], axis=1)).astype(bf16)
        in_maps.append(
            {"xtkv": xtkv, "wqt": wqt, "wkt": wkt, "wvt": wvt, "wot": wot, "bo": bo2}
        )
    return in_maps


def _run(x, Wq, Wk, Wv, Wo, bo, **spmd_kwargs):
    nc = _get_nc()
    in_maps = _make_in_maps(x, Wq, Wk, Wv, Wo, bo)
    res = run_bass_kernel_spmd(nc, in_maps, list(range(NCORES)), **spmd_kwargs)
    outs = [np.asarray(res.results[c]["out"]) for c in range(NCORES)]
    full = np.concatenate(outs, axis=0).reshape(4, 2048, D).astype(np.float32)
    return full, res


def kernel(x, Wq, Wk, Wv, Wo, bo):
    full, _ = _run(
        np.asarray(x), np.asarray(Wq), np.asarray(Wk), np.asarray(Wv),
        np.asarray(Wo), np.asarray(bo),
    )
    return full
